# revision 10
# baseline (speedup 1.0000x reference)
"""MultiHeadDiffAttention Trainium2 kernel (8 NeuronCores).

Sharding: data-parallel over batch (B=2 -> 2 groups of 4 cores), tensor-parallel
over heads within a group (16 heads -> 4 heads/core). Each core computes its
heads' attention output transposed [256, S]; per q-tile the group AllToAlls
the combined attention outputs (256KB vs 1MB for a Wo-partial ReduceScatter),
then every core applies the FULL Wo to its owned 128-row strip with fp32 PSUM
accumulation and LayerNorms it locally.

Schedule (v3):
- x/Wq/Wk/Wv/Wo shipped bf16; each loaded with few 3D-AP DMAs (dma_start
  triggers cost ~0.6us each on the Sync queue - many small DMAs serialize).
- Attention starts as soon as V, K(h0) and Q(h0,qt0) are projected
  (~35us); all remaining projections are drip-fed into the attention
  chunk loop as small interludes so the Scalar engine (exp) stays fed.
  Steady state is exp+tensor co-bound at ~1.0us per 128-k chunk.
- Attention loops qt-outer / h-inner; the AllToAll for q-tile qt fires at
  the qt boundary, and the local Wo matmul + LayerNorm for qt run as
  interludes inside q-tile qt+1's attention.
- PSUM: pa (A1|A2) has 3 bufs = 6 banks so A-pairs run a chunk ahead and
  exp issues back-to-back; projections/Wo cycle through the 2 po banks.
- LayerNorm rstd = exp(-0.5*ln(var+eps)): Ln and Exp share ACT table 6,
  so the Scalar engine never reloads activation tables mid-attention.
"""

import math
from contextlib import ExitStack

import numpy as np
import ml_dtypes

import concourse.bass as bass
import concourse.mybir as mybir
import concourse.tile as tile
from concourse import bacc
from concourse import bass_utils

F32 = mybir.dt.float32
BF16 = mybir.dt.bfloat16

B = 2
S = 2048
D = 1024
NH = 16
HD = 64
N_CORES = 8
NH_LOC = NH // (N_CORES // B)  # 4 heads per core
DQ = NH_LOC * 2 * HD  # 512 local q/k projection width
DV = NH_LOC * HD  # 256 local v projection width
LAYER_IDX = 12
LAMBDA_INIT = 0.8 - 0.6 * math.exp(-0.3 * (LAYER_IDX - 1))
LN_EPS = 1e-5
SCALE = HD ** (-0.5)

KC = D // 128  # 8 contraction chunks for projections
SB = S // 128  # 16 S-blocks / k-chunks
QT = 4  # q tiles of 512
GS = N_CORES // B  # 4-core tensor-parallel group

_CACHE = {}


def _build(single=False, nocc=False):
    nc = bacc.Bacc("TRN2", target_bir_lowering=False, debug=False,
                   num_devices=1 if single else N_CORES)

    xT = nc.dram_tensor("xT", [D, S], BF16, kind="ExternalInput").ap()
    wq = nc.dram_tensor("wq", [D, DQ], BF16, kind="ExternalInput").ap()
    wk = nc.dram_tensor("wk", [D, DQ], BF16, kind="ExternalInput").ap()
    wv = nc.dram_tensor("wv", [D, DV], BF16, kind="ExternalInput").ap()
    wo = nc.dram_tensor("wo", [DV, D], BF16, kind="ExternalInput").ap()
    lam = nc.dram_tensor("lam", [1, 1], F32, kind="ExternalInput").ap()
    gamma = nc.dram_tensor("gamma", [1, D], F32, kind="ExternalInput").ap()
    beta = nc.dram_tensor("beta", [1, D], F32, kind="ExternalInput").ap()
    ones4 = nc.dram_tensor("ones4", [1, NH_LOC], F32, kind="ExternalInput").ap()
    out = nc.dram_tensor("out", [S // 4, D], F32, kind="ExternalOutput").ap()

    with tile.TileContext(nc) as tc, ExitStack() as ctx:
        sb = ctx.enter_context(tc.tile_pool(name="sb", bufs=1))
        ps = ctx.enter_context(tc.tile_pool(name="ps", bufs=1, space="PSUM"))
        dram = ctx.enter_context(tc.tile_pool(name="dram", bufs=1, space="DRAM"))

        # All projection/Wo psum cycles through the "pa" tag (3 bufs, slots
        # recycle every chunk) so mid-chunk-loop interludes can never
        # deadlock against the live po1/po2 accumulators (tags pp0/pp1).
        def ps_tile(shape, name):
            return ps.tile(shape, F32, tag="pa", bufs=3, name=name)

        # ---- constants (gpsimd triggers, off the Sync path) ----
        lam_sb = sb.tile([1, 1], F32, tag="lam")
        nc.gpsimd.dma_start(out=lam_sb, in_=lam)
        gamma_sb = sb.tile([128, D], F32, tag="gamma")
        nc.gpsimd.dma_start(out=gamma_sb, in_=gamma.to_broadcast([128, D]))
        beta_sb = sb.tile([128, D], F32, tag="beta")
        nc.gpsimd.dma_start(out=beta_sb, in_=beta.to_broadcast([128, D]))
        ones4_sb = sb.tile([128, NH_LOC], F32, tag="ones4")
        nc.gpsimd.dma_start(out=ones4_sb, in_=ones4.to_broadcast([128, NH_LOC]))
        eps_sb = sb.tile([128, 1], F32, tag="eps")
        nc.vector.memset(eps_sb, LN_EPS)

        # ---- bulk inputs (order: wv, wk, x, wq, wo) ----
        wvc = sb.tile([128, KC, DV], BF16, tag="wv", name="wvc")
        nc.sync.dma_start(out=wvc, in_=wv.rearrange("(c p) m -> p c m", p=128))
        wkc = sb.tile([128, KC, DQ], BF16, tag="wk", name="wkc")
        nc.sync.dma_start(out=wkc, in_=wk.rearrange("(c p) m -> p c m", p=128))
        xall = sb.tile([128, KC, S], BF16, tag="x", name="xall")
        xsrc = xT.rearrange("(c p) s -> p c s", p=128)
        for q in range(QT):
            qs = slice(q * 512, (q + 1) * 512)
            nc.sync.dma_start(out=xall[:, :, qs], in_=xsrc[:, :, qs])
        wqc = sb.tile([128, KC, DQ], BF16, tag="wq", name="wqc")
        nc.sync.dma_start(out=wqc, in_=wq.rearrange("(c p) m -> p c m", p=128))
        woall = sb.tile([128, DV // 128, D], BF16, tag="wo", name="woall")
        nc.sync.dma_start(out=woall, in_=wo.rearrange("(c p) m -> p c m", p=128))

        # ---- persistent SBUF tiles ----
        vones = []
        for c in range(SB):
            t = sb.tile([128, NH_LOC, HD + 1], BF16, tag=f"vo{c}",
                        name=f"vones{c}")
            vones.append(t)
        kt = [sb.tile([128, S], BF16, tag=f"kt{h}", name=f"kt{h}")
              for h in range(NH_LOC)]
        qth = [sb.tile([128, S], BF16, tag=f"qt{h}", name=f"qth{h}")
               for h in range(NH_LOC)]
        otc = [sb.tile([128, S], BF16, tag=f"ot{c}", name=f"otc{c}")
               for c in range(2)]

        partial = [dram.tile([S // 4, D], BF16, name=f"partial{g}")
                   for g in range(QT)]
        red = [dram.tile([128, D], BF16, name=f"red{g}") for g in range(QT)]

        def vproj(qtb):
            for cl in range(4):
                c = qtb * 4 + cl
                pv = ps_tile([128, DV], "pv")
                for d in range(KC):
                    nc.tensor.matmul(pv, xall[:, d, c * 128:(c + 1) * 128],
                                     wvc[:, d, :], start=(d == 0),
                                     stop=(d == KC - 1))
                pvr = pv.rearrange("p (h v) -> p h v", h=NH_LOC)
                nc.vector.tensor_copy(vones[c][:, :, 0:HD], pvr)
                nc.vector.tensor_copy(
                    vones[c][:, :, HD:HD + 1],
                    ones4_sb.rearrange("p (a o) -> p a o", o=1))

        def kqproj(wc, dst, h, qtb):
            sl = slice(qtb * 512, (qtb + 1) * 512)
            hc0 = h * 128
            pk = ps_tile([128, 512], "pkq")
            for d in range(KC):
                nc.tensor.matmul(pk, wc[:, d, hc0:hc0 + 128], xall[:, d, sl],
                                 start=(d == 0), stop=(d == KC - 1))
            nc.vector.tensor_copy(dst[:, sl], pk)

        def ln_strip(g):
            """LayerNorm our owned strip of q-tile g from the RS output."""
            xb = sb.tile([128, D], BF16, tag="lnb", bufs=2, name="xb")
            nc.sync.dma_start(out=xb, in_=red[g][:, :])
            xt = sb.tile([128, D], F32, tag="lnx", bufs=2, name="xt")
            nc.vector.tensor_copy(xt, xb)
            xrr = xt.rearrange("p (a b) -> p a b", b=512)
            st = sb.tile([128, 2, 6], F32, tag="st", bufs=2, name="st")
            nc.vector.bn_stats(out=st[:, 0, :], in_=xrr[:, 0, :])
            nc.vector.bn_stats(out=st[:, 1, :], in_=xrr[:, 1, :])
            mv = sb.tile([128, 2], F32, tag="mv", bufs=2, name="mv")
            nc.vector.bn_aggr(out=mv, in_=st)
            # rstd = exp(-0.5*ln(var+eps)); Ln+Exp share ACT table 6
            lnv = sb.tile([128, 1], F32, tag="lnv", bufs=2, name="lnv")
            nc.scalar.activation(out=lnv, in_=mv[:, 1:2],
                                 func=mybir.ActivationFunctionType.Ln,
                                 bias=eps_sb, scale=1.0)
            rstd = sb.tile([128, 1], F32, tag="rstd", bufs=2, name="rstd")
            nc.scalar.activation(out=rstd, in_=lnv,
                                 func=mybir.ActivationFunctionType.Exp,
                                 scale=-0.5)
            ot = sb.tile([128, D], F32, tag="lno", bufs=2, name="ot")
            nc.vector.tensor_scalar(ot, xt, mv[:, 0:1], rstd,
                                    op0=mybir.AluOpType.subtract,
                                    op1=mybir.AluOpType.mult)
            nc.vector.tensor_mul(ot, ot, gamma_sb)
            nc.vector.tensor_add(ot, ot, beta_sb)
            nc.sync.dma_start(out=out[g * 128:(g + 1) * 128, :], in_=ot)

        # ---- minimal upfront projections ----
        for qtb in range(QT):
            vproj(qtb)
        for qtb in range(QT):
            kqproj(wkc, kt[0], 0, qtb)
        kqproj(wqc, qth[0], 0, 0)

        # Remaining projection / Wo+LN work drip-fed into the chunk loops.
        # Each task is (emit_fn,) emitted after a chunk's PV matmuls.
        def mk_kq(wc, dst, h, qtb):
            return lambda: kqproj(wc, dst, h, qtb)

        def mk_ln(g):
            return lambda: ln_strip(g)

        # ---- attention: qt-outer, h-inner ----
        for qt in range(QT):
            qsl = slice(qt * 512, (qt + 1) * 512)
            for h in range(NH_LOC):
                tasks = []
                if qt == 0 and h + 1 < NH_LOC:
                    # K and Q(qt0) for the next head
                    for qtb in range(QT):
                        tasks.append(mk_kq(wkc, kt[h + 1], h + 1, qtb))
                    tasks.append(mk_kq(wqc, qth[h + 1], h + 1, 0))
                if h >= 1 and qt + 1 < QT:
                    # Q(h-1, qt+1): ready a full head-block early
                    tasks.append(mk_kq(wqc, qth[h - 1], h - 1, qt + 1))
                if h == 0 and qt >= 1:
                    tasks.append(mk_kq(wqc, qth[3], 3, qt))
                if qt >= 1 and h == 3:
                    tasks.append(mk_ln(qt - 1))

                po1 = ps.tile([HD + 1, 512], F32, tag="pp0", bufs=1, name="po1")
                po2 = ps.tile([HD + 1, 512], F32, tag="pp1", bufs=1, name="po2")
                for c in range(SB):
                    ksl = slice(c * 128, (c + 1) * 128)
                    pa = ps.tile([128, 1024], F32, tag="pa", bufs=3, name="pa")
                    nc.tensor.matmul(pa[:, 0:512], kt[h][0:HD, ksl],
                                     qth[h][0:HD, qsl])
                    nc.tensor.matmul(pa[:, 512:1024], kt[h][HD:128, ksl],
                                     qth[h][HD:128, qsl])
                    e12 = sb.tile([128, 1024], BF16, tag="e12", bufs=4,
                                  name="e12")
                    nc.scalar.activation(out=e12, in_=pa,
                                         func=mybir.ActivationFunctionType.Exp,
                                         scale=SCALE)
                    nc.tensor.matmul(po1, vones[c][:, h, :], e12[:, 0:512],
                                     start=(c == 0), stop=(c == SB - 1))
                    nc.tensor.matmul(po2, vones[c][:, h, :], e12[:, 512:1024],
                                     start=(c == 0), stop=(c == SB - 1))
                    if tasks and c % 3 == 2:
                        tasks.pop(0)()
                # normalize: free psum first via sbuf copies
                cp1 = sb.tile([HD + 1, 512], F32, tag="cp1", bufs=2, name="cp1")
                cp2 = sb.tile([HD + 1, 512], F32, tag="cp2", bufs=2, name="cp2")
                nc.vector.tensor_copy(cp1, po1)
                nc.vector.tensor_copy(cp2, po2)
                for t in tasks:
                    t()
                sum1 = sb.tile([1, 512], F32, tag="u1", bufs=2, name="sum1")
                sum2 = sb.tile([1, 512], F32, tag="u2", bufs=2, name="sum2")
                nc.vector.tensor_copy(sum1, cp1[HD:HD + 1, :])
                nc.vector.tensor_copy(sum2, cp2[HD:HD + 1, :])
                s1 = sb.tile([1, 512], F32, tag="s1", bufs=2, name="s1")
                s2 = sb.tile([1, 512], F32, tag="s2", bufs=2, name="s2")
                nc.vector.reciprocal_approx_fast(out=s1, in_=sum1)
                nc.vector.reciprocal_approx_fast(out=s2, in_=sum2)
                nc.vector.tensor_scalar_mul(s2, s2, lam_sb[0:1, :])
                r1 = sb.tile([HD, 512], F32, tag="r1", bufs=2, name="r1")
                r2 = sb.tile([HD, 512], F32, tag="r2", bufs=2, name="r2")
                nc.gpsimd.partition_broadcast(r1, s1, channels=HD)
                nc.gpsimd.partition_broadcast(r2, s2, channels=HD)
                nc.vector.tensor_mul(r1, cp1[0:HD, :], r1)
                nc.vector.tensor_mul(r2, cp2[0:HD, :], r2)
                rb = (h % 2) * HD
                nc.vector.tensor_sub(otc[h // 2][rb:rb + HD, qsl], r1, r2)

            # Wo partial for this q-tile, then reduce-scatter it
            for sblk in range(4):
                csl = slice(qt * 512 + sblk * 128, qt * 512 + (sblk + 1) * 128)
                rsl = slice(sblk * 128, (sblk + 1) * 128)
                for ntile in range(2):
                    nsl = slice(ntile * 512, (ntile + 1) * 512)
                    pw = ps_tile([128, 512], "pw")
                    nc.tensor.matmul(pw, otc[0][:, csl], woall[:, 0, nsl],
                                     start=True, stop=False)
                    nc.tensor.matmul(pw, otc[1][:, csl], woall[:, 1, nsl],
                                     start=False, stop=True)
                    wout = sb.tile([128, 512], BF16, tag="wout", bufs=4,
                                   name="wout")
                    nc.vector.tensor_copy(wout, pw)
                    nc.sync.dma_start(out=partial[qt][rsl, nsl], in_=wout)
            if single or nocc:
                nc.sync.dma_start(out=red[qt][:, :], in_=partial[qt][0:128, :])
            else:
                nc.gpsimd.collective_compute(
                    "ReduceScatter",
                    mybir.AluOpType.add,
                    replica_groups=[[0, 1, 2, 3], [4, 5, 6, 7]],
                    ins=[partial[qt].opt()],
                    outs=[red[qt].opt()],
                )

        ln_strip(QT - 1)

    nc.compile()
    return nc


def _shard(inputs):
    x = np.asarray(inputs["x"], dtype=np.float32)
    Wq = np.asarray(inputs["Wq"], dtype=np.float32)
    Wk = np.asarray(inputs["Wk"], dtype=np.float32)
    Wv = np.asarray(inputs["Wv"], dtype=np.float32)
    Wo = np.asarray(inputs["Wo"], dtype=np.float32)
    gamma = np.asarray(inputs["gamma"], dtype=np.float32).reshape(1, D)
    beta = np.asarray(inputs["beta"], dtype=np.float32).reshape(1, D)
    lq1 = np.asarray(inputs["lambda_q1"], dtype=np.float32)
    lk1 = np.asarray(inputs["lambda_k1"], dtype=np.float32)
    lq2 = np.asarray(inputs["lambda_q2"], dtype=np.float32)
    lk2 = np.asarray(inputs["lambda_k2"], dtype=np.float32)
    lam = (np.exp(np.sum(lq1 * lk1, dtype=np.float32), dtype=np.float32)
           - np.exp(np.sum(lq2 * lk2, dtype=np.float32), dtype=np.float32)
           + np.float32(LAMBDA_INIT)).reshape(1, 1).astype(np.float32)
    ones4 = np.ones((1, NH_LOC), dtype=np.float32)

    bf = ml_dtypes.bfloat16
    wq_h = Wq.reshape(D, NH, 2 * HD)
    wk_h = Wk.reshape(D, NH, 2 * HD)
    wv_h = Wv.reshape(D, NH, HD)
    wo_h = Wo.reshape(NH, HD, D)

    xTs = [np.ascontiguousarray(x[b].T).astype(bf) for b in range(B)]
    in_maps = []
    for c in range(N_CORES):
        b = c // (N_CORES // B)
        hg = c % (N_CORES // B)
        hs = slice(hg * NH_LOC, (hg + 1) * NH_LOC)
        in_maps.append({
            "xT": xTs[b],
            "wq": np.ascontiguousarray(wq_h[:, hs, :].reshape(D, DQ)
                                       ).astype(bf),
            "wk": np.ascontiguousarray(wk_h[:, hs, :].reshape(D, DQ)
                                       ).astype(bf),
            "wv": np.ascontiguousarray(wv_h[:, hs, :].reshape(D, DV)
                                       ).astype(bf),
            "wo": np.ascontiguousarray(wo_h[hs].reshape(DV, D)).astype(bf),
            "lam": lam,
            "gamma": gamma,
            "beta": beta,
            "ones4": ones4,
        })
    return in_maps


def run_all(trace=False, nocc=False, **inputs):
    key = nocc
    if key not in _CACHE:
        _CACHE[key] = _build(nocc=nocc)
    nc = _CACHE[key]
    in_maps = _shard(inputs)
    res = bass_utils.run_bass_kernel_spmd(
        nc, in_maps, core_ids=list(range(N_CORES)), trace=trace)
    out = np.empty((B, S, D), dtype=np.float32)
    for c in range(N_CORES):
        b = c // (N_CORES // B)
        hg = c % (N_CORES // B)
        o = res.results[c]["out"]
        for g in range(QT):
            out[b, g * 512 + hg * 128: g * 512 + (hg + 1) * 128, :] = \
                o[g * 128:(g + 1) * 128, :]
    return out, res


def kernel(**inputs):
    out, _ = run_all(trace=False, **inputs)
    return out
